# revision 17
# baseline (speedup 1.0000x reference)
"""Causal-free attention kernel for Trainium2 (8 NeuronCores), v2.

Model (per batch b):
  q/k/v = x @ W{q,k,v}.T + b{q,k,v}            [S, D] -> heads [H, S, 64]
  scoresT[h, sk, sq] = (k_h q_h^T)/8 ; softmax over sk with key-bias
      tw*treatment[b, sk] (confounder bias is constant over the softmax
      axis and cancels)
  out = attn @ v -> merge heads -> @ Wo.T + bo

Sharding: core c -> batch c//4, head-group c%4 (4 heads, 256 dims of the
qkv/out projections). Each core computes its partial of the final
projection; host sums the 4 partials per batch and adds bo.

v2 structure (vs v1): the two heads of a pair are interleaved per key
chunk j so their K=64 score matmuls sit adjacent in the PE stream
(row groups 0-1 vs 2-3 -> concurrent on HW); both heads' p*v
accumulate into one 3-bank PSUM tile; projections stream in as
deadline-ordered fillers so the Act exp stream starts early and never
starves; DMA is spread across 4 queues; copies move to GpSimd.
"""

import numpy as np

B, S, D, H, HD = 2, 2048, 1024, 16, 64
N_CORES = 8
GROUPS = 4          # head-groups per batch
GD = D // GROUPS    # 256 outdims per group
KC = D // 128       # 8 contraction chunks
NT = S // 128       # 16 token chunks
JC = S // 128       # 16 key chunks
PANEL = 1024        # sq panel width
NPAN = S // PANEL   # 2 panels

# pv slot packing: 16 slots (2 heads x 8 sq-chunks) of 65 f32, 7 per
# 512-f32 PSUM bank so no matmul output crosses a bank boundary.
PVOFF = [(s // 7) * 512 + (s % 7) * 65 for s in range(16)]

_CACHE = {}


def _build_nc(do_compile=True, iters=1):
    import concourse.bass as bass  # noqa: F401
    import concourse.mybir as mybir
    import concourse.tile as tile
    from concourse import bacc
    from concourse.masks import make_identity
    from contextlib import ExitStack

    dt = mybir.dt
    f32, bf16 = dt.float32, dt.bfloat16
    AF = mybir.ActivationFunctionType

    nc = bacc.Bacc()

    xt = nc.declare_dram_parameter("xt", [D, S], bf16, isOutput=False)
    wq = nc.declare_dram_parameter("wq", [D, GD], bf16, isOutput=False)
    wk = nc.declare_dram_parameter("wk", [D, GD], bf16, isOutput=False)
    wv = nc.declare_dram_parameter("wv", [D, GD], bf16, isOutput=False)
    wo = nc.declare_dram_parameter("wo", [GD, D], bf16, isOutput=False)
    bq = nc.declare_dram_parameter("bq", [128, 2], f32, isOutput=False)
    bk = nc.declare_dram_parameter("bk", [128, 2], f32, isOutput=False)
    bv = nc.declare_dram_parameter("bv", [1, GD], bf16, isOutput=False)
    tb = nc.declare_dram_parameter("tb", [128, JC], f32, isOutput=False)
    out = nc.declare_dram_parameter("out", [S, D], bf16, isOutput=True)

    with tile.TileContext(nc) as tc, ExitStack() as ctx:
        sing = ctx.enter_context(tc.tile_pool(name="sing", bufs=1))
        apool = ctx.enter_context(tc.tile_pool(name="apool", bufs=6))
        dpool = ctx.enter_context(tc.tile_pool(name="dpool", bufs=2))
        obpool = ctx.enter_context(tc.tile_pool(name="obpool", bufs=6))
        psc = ctx.enter_context(tc.tile_pool(name="psc", bufs=1, space="PSUM"))
        ppv = ctx.enter_context(tc.tile_pool(name="ppv", bufs=1, space="PSUM"))
        psm = ctx.enter_context(tc.tile_pool(name="psm", bufs=1, space="PSUM"))
        if iters > 1:
            ctx.enter_context(tc.For_i(
                0, iters, 1,
                hint_engines=(
                    mybir.EngineType.PE,
                    mybir.EngineType.Activation,
                    mybir.EngineType.DVE,
                    mybir.EngineType.SP,
                    mybir.EngineType.Pool,
                )))

        # ---- constants
        ident = sing.tile([128, 128], f32, tag="ident", name="ident")
        make_identity(nc, ident)

        xt3 = xt.rearrange("(c p) t -> c p t", p=128)
        # weights as single 3D DMAs
        wq3 = wq.rearrange("(c p) m -> p c m", p=128)
        wk3 = wk.rearrange("(c p) m -> p c m", p=128)
        wv3 = wv.rearrange("(c p) m -> p c m", p=128)
        wo3 = wo.rearrange("(c p) m -> p c m", p=128)

        # ---- DMA loads, spread over 4 queues. Priority: xt col-group 0
        # (all 8 contraction chunks) + wk + wq first so the k/q projection
        # prelude can start ~1.6us in.
        xt_t = [sing.tile([128, S], bf16, tag=f"xt{k}", name=f"xt{k}")
                for k in range(KC)]
        wq_sb = sing.tile([128, KC, GD], bf16, tag="wq", name="wq")
        wk_sb = sing.tile([128, KC, GD], bf16, tag="wk", name="wk")
        wv_sb = sing.tile([128, KC, GD], bf16, tag="wv", name="wv")
        wo_sb = sing.tile([128, 2, D], bf16, tag="wo", name="wo")

        # DMA queues: SP, Pool (gpsimd), Act (prelude only — Act is the
        # bottleneck engine once the exp stream starts). Priority order:
        # biases first on Act (tiny, needed at the end of the first k/q
        # chains), wk half 0 + xt col-group 0 feed the k-proj prelude.
        bq_sb = sing.tile([128, 2], f32, tag="bq", name="bq")
        bk_sb = sing.tile([128, 2], f32, tag="bk", name="bk")
        bv_sb = sing.tile([1, GD], bf16, tag="bv", name="bv")
        tb_sb = sing.tile([128, JC], f32, tag="tb", name="tb")
        nc.scalar.dma_start(bk_sb[:], bk[:])
        nc.scalar.dma_start(bq_sb[:], bq[:])
        nc.scalar.dma_start(bv_sb[:], bv[:])
        nc.scalar.dma_start(tb_sb[:], tb[:])
        nc.gpsimd.dma_start(wk_sb[:, :, 0:128], wk3[:, :, 0:128])
        nc.sync.dma_start(wk_sb[:, :, 128:256], wk3[:, :, 128:256])
        for k in range(KC):
            eng = (nc.sync, nc.gpsimd, nc.scalar)[k % 3]
            eng.dma_start(xt_t[k][:, 0:512], xt3[k][:, 0:512])
        nc.sync.dma_start(wq_sb[:, :, 0:128], wq3[:, :, 0:128])
        nc.gpsimd.dma_start(wq_sb[:, :, 128:256], wq3[:, :, 128:256])
        nc.scalar.dma_start(wv_sb[:], wv3)
        for g in range(1, 4):
            cols = slice(g * 512, (g + 1) * 512)
            for k in range(KC):
                eng = (nc.sync, nc.gpsimd, nc.scalar)[(k + g) % 3]
                eng.dma_start(xt_t[k][:, cols], xt3[k][:, cols])
        nc.gpsimd.dma_start(wo_sb[:], wo3)
        ones_sb = sing.tile([1, 128], bf16, tag="ones", name="ones")
        nc.vector.memset(ones_sb[:], 1.0)

        qT = [sing.tile([128, S], bf16, tag=f"qT{p}", name=f"qT{p}") for p in range(2)]
        kT = [sing.tile([128, S], bf16, tag=f"kT{p}", name=f"kT{p}") for p in range(2)]
        v_sb = [sing.tile([128, JC, 130], bf16, tag=f"v{p}", name=f"v{p}") for p in range(2)]
        ot = [sing.tile([128, S], bf16, tag=f"ot{p}", name=f"ot{p}") for p in range(2)]
        op = [sing.tile([128, NT, 128], f32, tag=f"op{p}", name=f"op{p}") for p in range(2)]

        for p in range(2):
            nc.gpsimd.memset(v_sb[p][:, :, 64:65], 1.0)
            nc.gpsimd.memset(v_sb[p][:, :, 129:130], 1.0)

        # ---- phase generators
        def proj_qk_steps(pair, w_sb, b_sb, dest, n0, n1, slot=None):
            # one 512-token group of the q/k projection per n. `slot`
            # optionally borrows a psc tag (prelude pipelining only).
            for n in range(n0, n1):
                if slot is None:
                    ps = psm.tile([128, 512], f32, tag="sm", name="sm")
                else:
                    ps = psc.tile([128, 1024], f32, tag=slot, name=slot)
                    ps = ps[:, 0:512]
                for k in range(KC):
                    nc.tensor.matmul(
                        ps[:],
                        w_sb[:, k, pair * 128:(pair + 1) * 128],
                        xt_t[k][:, n * 512:(n + 1) * 512],
                        start=(k == 0), stop=(k == KC - 1),
                    )
                    if k % 2 == 1:
                        yield
                nc.vector.tensor_scalar_add(
                    dest[:, n * 512:(n + 1) * 512], ps[:],
                    b_sb[:, pair:pair + 1])
                yield

        def proj_v_steps(pair, c0=0, c1=NT):
            # one token chunk of the v projection per step
            cols = slice(pair * 128, (pair + 1) * 128)
            for mt in range(c0, c1):
                ps = psm.tile([128, 512], f32, tag="sm", name="sm")
                for k in range(KC):
                    nc.tensor.matmul(
                        ps[:, 0:128],
                        xt_t[k][:, mt * 128:(mt + 1) * 128],
                        wv_sb[:, k, cols],
                        start=(k == 0), stop=False,
                    )
                nc.tensor.matmul(
                    ps[:, 0:128], ones_sb[:], bv_sb[:, cols],
                    start=False, stop=True,
                )
                dst = v_sb[pair][:, mt].rearrange(
                    "p (h c) -> p h c", c=65)[:, :, 0:64]
                src = ps[:, 0:128].rearrange("p (h c) -> p h c", c=64)
                nc.vector.tensor_copy(out=dst, in_=src)
                yield

        def transpose_steps(pair, panel, tail=False):
            for cq in range(panel * (PANEL // 128), (panel + 1) * (PANEL // 128)):
                pt = psm.tile([128, 512], f32, tag="sm", name="sm")
                nc.tensor.transpose(pt[:, 0:128], op[pair][:, cq, :], ident[:])
                dst = ot[pair][:, cq * 128:(cq + 1) * 128]
                if tail and cq % 2 == 1:
                    nc.scalar.copy(dst, pt[:, 0:128])
                else:
                    nc.vector.tensor_copy(out=dst, in_=pt[:, 0:128])
                yield

        def out_proj_steps(mt0, mt1, alt_pool=False):
            for mt in range(mt0, mt1):
                for n in range(2):
                    if alt_pool and n == 1:
                        pf = psc.tile([128, 1024], f32, tag="sc0", name="sc0")
                        pf = pf[:, 0:512]
                    else:
                        pf = psm.tile([128, 512], f32, tag="sm", name="sm")
                        pf = pf[:]
                    for pair in range(2):
                        nc.tensor.matmul(
                            pf,
                            ot[pair][:, mt * 128:(mt + 1) * 128],
                            wo_sb[:, pair, n * 512:(n + 1) * 512],
                            start=(pair == 0), stop=(pair == 1),
                        )
                    ob = obpool.tile([128, 512], bf16, tag="ob", name="ob")
                    if alt_pool and n == 1:
                        nc.scalar.copy(ob[:], pf)
                    else:
                        nc.vector.tensor_copy(out=ob[:], in_=pf)
                    (nc.gpsimd if n == 0 else nc.sync).dma_start(
                        out[mt * 128:(mt + 1) * 128, n * 512:(n + 1) * 512],
                        ob[:])
                    yield

        def chain(*gens):
            for g in gens:
                yield from g

        def drain(g):
            for _ in g:
                pass

        def wrr(pattern, *gens):
            """Weighted round-robin merge: cycles `pattern` (generator
            indices), yielding one real step per pump; exhausted generators
            are skipped without consuming a pump."""
            gens = [iter(g) for g in gens]
            alive = [True] * len(gens)
            while any(alive):
                for idx in pattern:
                    if alive[idx]:
                        try:
                            next(gens[idx])
                            yield
                        except StopIteration:
                            alive[idx] = False

        def attention_pair(pair, panel, filler):
            """Both heads of `pair`, queries [panel*PANEL, (panel+1)*PANEL).

            Software-pipelined: per key chunk j and head hh we emit [filler,
            pv of (j-1, hh), scores of (j, hh)]. The pv MMs and the score
            MMs are both gated on exp_hh(j-1) completing (data dep and sc
            WAR respectively), so the PE never sits on a long sem wait with
            ready work behind it, and the Act engine never starves."""
            q0 = panel * PANEL
            pv = ppv.tile([128, 1536], f32, tag="pv", name="pv")
            nc.vector.memset(pv[:], 0.0)

            def emit_pv(at, hh, jv):
                for i in range(PANEL // 128):
                    off = PVOFF[hh * 8 + i]
                    nc.tensor.matmul(
                        pv[:, off:off + 65],
                        at[:, i * 128:(i + 1) * 128],
                        v_sb[pair][:, jv, hh * 65:(hh + 1) * 65],
                        start=False, stop=False,
                        skip_group_check=True,
                    )

            prev = None  # (ats, j) of the previous key chunk
            for j in range(JC):
                jc = slice(j * 128, (j + 1) * 128)
                scs = []
                for hh in range(2):
                    sc = psc.tile([128, PANEL], f32, tag=f"sc{hh}",
                                  name=f"sc{hh}")
                    rows = slice(hh * 64, (hh + 1) * 64)
                    for n2 in range(PANEL // 512):
                        nc.tensor.matmul(
                            sc[:, n2 * 512:(n2 + 1) * 512],
                            kT[pair][rows, jc],
                            qT[pair][rows,
                                     q0 + n2 * 512:q0 + (n2 + 1) * 512],
                            start=True, stop=True,
                        )
                    scs.append(sc)
                    if prev is not None:
                        emit_pv(prev[0][hh], hh, prev[1])
                    next(filler, None)
                    if hh == 1:
                        next(filler, None)
                ats = []
                for hh in range(2):
                    at = apool.tile([128, PANEL], bf16, tag="at", name="at")
                    nc.scalar.activation(
                        at[:], scs[hh][:], AF.Exp,
                        bias=tb_sb[:, j:j + 1], scale=0.125)
                    ats.append(at)
                prev = (ats, j)
            for hh in range(2):
                next(filler, None)
                emit_pv(prev[0][hh], hh, prev[1])
            # denominators (pv col 64 of each slot) and normalization
            den = dpool.tile([128, 16], f32, tag="den", name="den")
            pv0 = pv[:, 0:455].rearrange("p (s c) -> p s c", c=65)
            pv1 = pv[:, 512:967].rearrange("p (s c) -> p s c", c=65)
            pv2 = pv[:, 1024:1154].rearrange("p (s c) -> p s c", c=65)
            nc.vector.reciprocal(den[:, 0:7], pv0[:, :, 64])
            nc.vector.reciprocal(den[:, 7:14], pv1[:, :, 64])
            nc.vector.reciprocal(den[:, 14:16], pv2[:, :, 64])
            for hh in range(2):
                for i in range(PANEL // 128):
                    s = hh * 8 + i
                    cq = panel * (PANEL // 128) + i
                    nc.vector.tensor_scalar_mul(
                        op[pair][:, cq, hh * 64:(hh + 1) * 64],
                        pv[:, PVOFF[s]:PVOFF[s] + 64],
                        den[:, s:s + 1])

        # ---- prelude: minimum work before (pair0, panel0) attention:
        # k0 tokens 0-511, q0 panel0 (tokens 0-1023), v0 c0-1. The q
        # chains borrow the (still unused) sc PSUM slots so the prelude
        # chains pipeline instead of serializing on the single psm slot.
        drain(proj_qk_steps(0, wk_sb, bk_sb, kT[0], 0, 1))
        drain(proj_qk_steps(0, wq_sb, bq_sb, qT[0], 0, 1, slot="sc0"))
        drain(proj_qk_steps(0, wq_sb, bq_sb, qT[0], 1, 2, slot="sc1"))
        drain(proj_v_steps(0, 0, 2))

        # (pair0, panel0) fillers. Deadlines: k0 n1/n2/n3 by j4/j8/j12,
        # q0 n2-3 by panel1, v0 c_j by ~j (soft; pv may lag).
        hard0 = chain(
            proj_qk_steps(0, wk_sb, bk_sb, kT[0], 1, 4),
            proj_qk_steps(0, wq_sb, bq_sb, qT[0], 2, 4),
        )
        soft0 = proj_v_steps(0, 2, NT)
        f0 = wrr([0, 0, 1], hard0, soft0)
        attention_pair(0, 0, f0)

        # (pair0, panel1): leftovers + pair1 prelude-equivalent:
        # k1 n0, q1 n0-1, v1 c0-2 (all needed at (pair1, panel0) start).
        hard1 = chain(
            f0,
            proj_qk_steps(1, wk_sb, bk_sb, kT[1], 0, 1),
            proj_qk_steps(1, wq_sb, bq_sb, qT[1], 0, 2),
            proj_v_steps(1, 0, 3),
        )
        f1 = wrr([0, 0, 0, 1], hard1, transpose_steps(0, 0))
        attention_pair(0, 1, f1)

        # (pair1, panel0): k1 n1-3, q1 n2-3 hard; v1 c3.., transposes soft.
        hard2 = chain(
            f1,
            proj_qk_steps(1, wk_sb, bk_sb, kT[1], 1, 4),
            proj_qk_steps(1, wq_sb, bq_sb, qT[1], 2, 4),
        )
        soft2 = chain(proj_v_steps(1, 3, NT), transpose_steps(0, 1))
        f2 = wrr([0, 0, 1], hard2, soft2)
        attention_pair(1, 0, f2)

        # (pair1, panel1): leftovers + transposes of (1,0) + out proj of
        # token chunks 0-7 (panel0 tokens, both pairs now done).
        f3 = chain(f2, transpose_steps(1, 0), out_proj_steps(0, 8))
        attention_pair(1, 1, f3)
        drain(f3)

        # tail: transposes of (1,1) then out proj of token chunks 8-15,
        # alternating PSUM slots (sc pool is free now) to pipeline.
        drain(transpose_steps(1, 1, tail=True))
        drain(out_proj_steps(8, NT, alt_pool=True))

    if do_compile:
        nc.compile()
    return nc


def _get_nc():
    if "nc" not in _CACHE:
        _CACHE["nc"] = _build_nc()
    return _CACHE["nc"]


def _host_shard(inputs):
    import ml_dtypes

    bf = ml_dtypes.bfloat16
    f = np.float32
    x = np.asarray(inputs["x"], f)
    treatment = np.asarray(inputs["treatment"], f)
    Wq = np.asarray(inputs["Wq"], f)
    Wk = np.asarray(inputs["Wk"], f)
    Wv = np.asarray(inputs["Wv"], f)
    Wo = np.asarray(inputs["Wo"], f)
    bq = np.asarray(inputs["bq"], f)
    bk = np.asarray(inputs["bk"], f)
    bv = np.asarray(inputs["bv"], f)
    tw = float(np.asarray(inputs["treatment_weight"], f)[0])

    C = np.ascontiguousarray
    in_maps = []
    for c in range(N_CORES):
        b, g = c // GROUPS, c % GROUPS
        o0 = g * GD
        in_maps.append({
            "xt": C(x[b].T).astype(bf),
            "wq": C(Wq[o0:o0 + GD, :].T).astype(bf),
            "wk": C(Wk[o0:o0 + GD, :].T).astype(bf),
            "wv": C(Wv[o0:o0 + GD, :].T).astype(bf),
            "wo": C(Wo[:, o0:o0 + GD].T).astype(bf),
            "bq": C(bq[o0:o0 + GD].reshape(2, 128).T),
            "bk": C(bk[o0:o0 + GD].reshape(2, 128).T),
            "bv": C(bv[o0:o0 + GD].reshape(1, GD)).astype(bf),
            "tb": C((tw * treatment[b]).reshape(JC, 128).T),
        })
    return in_maps


def _host_gather(results, inputs):
    bo = np.asarray(inputs["bo"], np.float32)
    outs = []
    for b in range(B):
        acc = np.zeros((S, D), np.float32)
        for g in range(GROUPS):
            acc += np.asarray(results[b * GROUPS + g]["out"]).astype(np.float32)
        outs.append(acc + bo[None, :])
    return np.stack(outs).astype(np.float32)


def kernel(**inputs):
    from concourse.bass_utils import run_bass_kernel_spmd

    nc = _get_nc()
    in_maps = _host_shard(inputs)
    res = run_bass_kernel_spmd(nc, in_maps, list(range(N_CORES)))
    return _host_gather(res.results, inputs)


def run_traced(inputs, **kw):
    """Test helper: same as kernel() but returns (output, BassKernelResults)."""
    from concourse.bass_utils import run_bass_kernel_spmd

    nc = _get_nc()
    in_maps = _host_shard(inputs)
    res = run_bass_kernel_spmd(nc, in_maps, list(range(N_CORES)), **kw)
    return _host_gather(res.results, inputs), res
